# revision 13
# baseline (speedup 1.0000x reference)
"""ChannelWiseMamba Trainium2 kernel (8-core SPMD, data-parallel over batch*channels).

Layout strategy:
  - 32 sequences (batch*channels fused) -> 4 per core, weights replicated.
  - Host pre-transposes x to [seq, d_model, L] (f16) so matmul rhs loads need
    no on-device transpose; output is produced as [seq, d_model, L] f32 and
    transposed back on host.
  - On-device per sequence:
      in_proj (PE f16) -> depthwise conv (PE f32, diagonal-matmul taps)
      -> silu (ACT tanh form: silu(v)=v*(0.5*tanh(v/2)+0.5)) -> x_proj (PE
      f16) -> dt_proj + softplus (PE + ACT exp/ln) -> selective scan:
         dA_s = exp(A[:,s]*dt) on ACT (per-partition scale AP),
         u = (dt*x)*B (DVE f16 2x), recurrence via tensor_tensor_scan (DVE,
         fp32 state), y = sum_s C*h via f16 multiply + pair tree,
      then y2 = y + D*xs, gate with z*sigmoid(z), out_proj (PE f16).
  - Scan state layout: [channel partitions, (s,t) free]; the scan chains all
    s-runs in one instruction, with dA zeroed at each s-run start so the
    state resets (h0 = 0) without per-run initials.
  - Scan loop nest is qg-outer / cb-inner so the broadcast B/C tiles for one
    d_state group are fully consumed before the next group's are needed
    (avoids tile-pool slot deadlock). dt/dtx/xs are spilled to DRAM per cb
    and re-loaded transiently.
"""

import os
import numpy as np

import concourse.bass as bass
import concourse.mybir as mybir
import concourse.tile as tile
from concourse import bacc
from concourse.bass_utils import run_bass_kernel_spmd

F16 = mybir.dt.float16
F32 = mybir.dt.float32
OP = mybir.AluOpType
AF = mybir.ActivationFunctionType

BATCH, CH, SEQ_L, D_MODEL = 4, 8, 1024, 512
D_STATE, D_CONV, EXPAND = 16, 4, 2
D_INNER = EXPAND * D_MODEL          # 1024
DT_RANK = (D_MODEL + 15) // 16      # 32
N_CORES = 8
NSEQ = (BATCH * CH) // N_CORES      # 4 per core
NCB = D_INNER // 128                # 8 channel blocks
NKM = D_MODEL // 128                # 4 k-tiles over d_model
SG = 4                              # d_state group size for scan tiles
NQG = D_STATE // SG

LAST_EXEC_NS = None
LAST_PROFILE = None


def _bcast_ap(src_ap, width):
    """Partition-broadcast read AP: replicate a linear DRAM range to 128 rows."""
    return bass.AP(tensor=src_ap.tensor, offset=src_ap.offset,
                   ap=[[0, 128], [1, width]])


def build_program(nseq=NSEQ, L=SEQ_L):
    TCH = min(512, L)
    NCH = L // TCH
    assert L % TCH == 0

    nc = bacc.Bacc("TRN2", target_bir_lowering=False)

    xT16 = nc.dram_tensor("xT16", [nseq, 128, NKM * L], F16, kind="ExternalInput")
    winT16 = nc.dram_tensor("winT16", [128, NKM * 2 * D_INNER], F16, kind="ExternalInput")
    wxT16 = nc.dram_tensor("wxT16", [128, NCB * 64], F16, kind="ExternalInput")
    wdtT16 = nc.dram_tensor("wdtT16", [DT_RANK, D_INNER], F16, kind="ExternalInput")
    woutT16 = nc.dram_tensor("woutT16", [128, NCB * D_MODEL], F16, kind="ExternalInput")
    cdiag = nc.dram_tensor("cdiag", [128, D_CONV * NCB * 128], F32, kind="ExternalInput")
    # packed per-channel columns: A (16*NCB) then bdt, conv_b/2, conv_b, D
    smallw = nc.dram_tensor("smallw", [128, NCB * D_STATE + 4 * NCB], F32,
                            kind="ExternalInput")
    outT = nc.dram_tensor("outT", [nseq, D_MODEL, L], F32, kind="ExternalOutput")

    with tile.TileContext(nc) as tc:
        import contextlib
        with contextlib.ExitStack() as ctx:
            const = ctx.enter_context(tc.tile_pool(name="const", bufs=1))
            xtp = ctx.enter_context(tc.tile_pool(name="xtp", bufs=2))
            ph1 = ctx.enter_context(tc.tile_pool(name="ph1", bufs=2))
            reload_p = ctx.enter_context(tc.tile_pool(name="reload", bufs=2))
            scan2 = ctx.enter_context(tc.tile_pool(name="scan2", bufs=2))
            scan1 = ctx.enter_context(tc.tile_pool(name="scan1", bufs=1))
            bcp = ctx.enter_context(tc.tile_pool(name="bcp", bufs=2))
            yaccp = ctx.enter_context(tc.tile_pool(name="yaccp", bufs=1))
            yp = ctx.enter_context(tc.tile_pool(name="yp", bufs=2))
            tailp = ctx.enter_context(tc.tile_pool(name="tailp", bufs=1))
            dramp = ctx.enter_context(tc.tile_pool(name="dramp", bufs=2, space="DRAM"))
            ps_mm = ctx.enter_context(tc.tile_pool(name="ps_mm", bufs=4, space="PSUM"))
            ps_x = ctx.enter_context(tc.tile_pool(name="ps_x", bufs=2, space="PSUM"))
            ps_o = ctx.enter_context(tc.tile_pool(name="ps_o", bufs=2, space="PSUM"))

            # ---- static weights ----
            wT_sb = const.tile([128, NKM * 2 * D_INNER], F16)
            nc.sync.dma_start(wT_sb[:, :], winT16[:, :])
            wx_sb = const.tile([128, NCB * 64], F16)
            nc.sync.dma_start(wx_sb[:, :], wxT16[:, :])
            wdt_sb = const.tile([128, D_INNER], F16)
            nc.sync.dma_start(wdt_sb[0:DT_RANK, :], wdtT16[:, :])
            wout_sb = const.tile([128, NCB * D_MODEL], F16)
            nc.sync.dma_start(wout_sb[:, :], woutT16[:, :])
            diag_sb = const.tile([128, D_CONV * NCB * 128], F32)
            nc.sync.dma_start(diag_sb[:, :], cdiag[:, :])
            sw_sb = const.tile([128, NCB * D_STATE + 4 * NCB], F32)
            nc.sync.dma_start(sw_sb[:, :], smallw[:, :])
            A_sb = sw_sb[:, 0:NCB * D_STATE]
            o = NCB * D_STATE
            bdt_sb = sw_sb[:, o:o + NCB]
            cbh_sb = sw_sb[:, o + NCB:o + 2 * NCB]
            cb_sb = sw_sb[:, o + 2 * NCB:o + 3 * NCB]
            d_sb = sw_sb[:, o + 3 * NCB:o + 4 * NCB]

            def lhs_in(k, m):
                return wT_sb[:, k * 2 * D_INNER + m * 128: k * 2 * D_INNER + m * 128 + 128]

            for seq in range(nseq):
                xT_sb = xtp.tile([128, NKM * L], F16, tag="xT_sb", name=f"xT_sb_{seq}")
                nc.sync.dma_start(xT_sb[:, :], xT16[seq, :, :])

                xps = [ps_x.tile([64, TCH], F32, tag="xps", name=f"xps_{seq}_{i}")
                       for i in range(NCH)]
                dtD, dtxD, xsD = [], [], []
                for cb in range(NCB):
                    # ---- in_proj x-half -> padded raw tile ----
                    xsraw = ph1.tile([128, L], F32, tag="xsraw", name=f"xsraw_{seq}_{cb}")
                    for chk in range(NCH):
                        ps = ps_mm.tile([128, TCH], F32, tag="ps_mm", name=f"ps_{seq}_{cb}_{chk}")
                        for k in range(NKM):
                            nc.tensor.matmul(ps[:, :], lhs_in(k, cb),
                                             xT_sb[:, k * L + chk * TCH: k * L + (chk + 1) * TCH],
                                             start=(k == 0), stop=(k == NKM - 1))
                        nc.scalar.copy(xsraw[:, chk * TCH: (chk + 1) * TCH], ps[:, :])
                    # ---- depthwise conv via diagonal matmuls + silu ----
                    thxc = ph1.tile([128, 2 * L], F16, tag="thxc", name=f"thxc_{seq}_{cb}")
                    th = thxc[:, 0:L]
                    xc = thxc[:, L:2 * L]
                    for chk in range(NCH):
                        cps = ps_mm.tile([128, TCH], F32, tag="ps_mm", name=f"cps_{seq}_{cb}_{chk}")
                        for j in range(D_CONV - 1, -1, -1):
                            # j = D_CONV-1 first: full width, starts the psum
                            # group; earlier taps are shortened at the causal
                            # edge and accumulate into the already-written bank
                            dslice = diag_sb[:, (j * NCB + cb) * 128: (j * NCB + cb) * 128 + 128]
                            off = chk * TCH - (D_CONV - 1) + j
                            if off < 0:
                                nc.tensor.matmul(cps[:, -off:TCH], dslice,
                                                 xsraw[:, 0:TCH + off],
                                                 start=False, stop=(j == 0))
                            else:
                                nc.tensor.matmul(cps[:, :], dslice,
                                                 xsraw[:, off:off + TCH],
                                                 start=(j == D_CONV - 1), stop=(j == 0))
                        nc.scalar.activation(th[:, chk * TCH:(chk + 1) * TCH], cps[:, :],
                                             AF.Tanh, bias=cbh_sb[:, cb:cb + 1], scale=0.5)
                        nc.scalar.activation(xc[:, chk * TCH:(chk + 1) * TCH], cps[:, :],
                                             AF.Identity, bias=cb_sb[:, cb:cb + 1], scale=1.0)
                    nc.gpsimd.tensor_scalar(th, th, 0.5, 0.5, OP.mult, OP.add)
                    xs16 = ph1.tile([128, L], F16, tag="xs16", name=f"xs16_{seq}_{cb}")
                    nc.gpsimd.tensor_tensor(xs16[:, :], xc, th, OP.mult)
                    # ---- x_proj partial (K accumulation over channel blocks) ----
                    for chk in range(NCH):
                        nc.tensor.matmul(xps[chk][:, :], wx_sb[:, cb * 64:(cb + 1) * 64],
                                         xs16[:, chk * TCH:(chk + 1) * TCH],
                                         start=(cb == 0), stop=(cb == NCB - 1))
                    xsd = dramp.tile([128, L], F16, tag=f"xsD{cb}", name=f"xsD_{seq}_{cb}")
                    nc.sync.dma_start(xsd[:, :], xs16[:, :])
                    xsD.append(xsd)

                # ---- x_dbl evacuation + B/C spill ----
                xdbl16 = xtp.tile([128, L], F16, tag="xdbl16", name=f"xdbl_{seq}")
                for chk in range(NCH):
                    nc.scalar.copy(xdbl16[0:64, chk * TCH:(chk + 1) * TCH], xps[chk][:, :])
                bcd = dramp.tile([32, L], F16, tag="bcD", name=f"bcD_{seq}")
                nc.sync.dma_start(bcd[:, :], xdbl16[32:64, :])

                # ---- dt_proj + softplus; spill dt and dtx per cb ----
                for cb in range(NCB):
                    dt16 = ph1.tile([128, L], F16, tag="dt16", name=f"dt16_{seq}_{cb}")
                    for chk in range(NCH):
                        dps = ps_mm.tile([128, TCH], F32, tag="ps_mm", name=f"dps_{seq}_{cb}_{chk}")
                        nc.tensor.matmul(dps[:, :], wdt_sb[0:DT_RANK, cb * 128:(cb + 1) * 128],
                                         xdbl16[0:DT_RANK, chk * TCH:(chk + 1) * TCH],
                                         start=True, stop=True)
                        spe = ph1.tile([128, TCH], F16, tag="spe", name=f"spe_{seq}_{cb}_{chk}")
                        nc.scalar.activation(spe[:, :], dps[:, :], AF.Exp,
                                             bias=bdt_sb[:, cb:cb + 1], scale=1.0)
                        nc.scalar.activation(dt16[:, chk * TCH:(chk + 1) * TCH], spe[:, :],
                                             AF.Ln, bias=1.0, scale=1.0)
                    dtd = dramp.tile([128, L], F16, tag=f"dtD{cb}", name=f"dtD_{seq}_{cb}")
                    nc.sync.dma_start(dtd[:, :], dt16[:, :])
                    dtD.append(dtd)
                    xs16r0 = reload_p.tile([128, L], F16, tag="xs16r0", name=f"xs16r0_{seq}_{cb}")
                    nc.sync.dma_start(xs16r0[:, :], xsD[cb][:, :])
                    dtx16 = ph1.tile([128, L], F16, tag="dtx16", name=f"dtx_{seq}_{cb}")
                    nc.vector.tensor_tensor(dtx16[:, :], dt16[:, :], xs16r0[:, :], OP.mult)
                    dtxd = dramp.tile([128, L], F16, tag=f"dtxD{cb}", name=f"dtxD_{seq}_{cb}")
                    nc.sync.dma_start(dtxd[:, :], dtx16[:, :])
                    dtxD.append(dtxd)

                # ---- scan block: qg outer, cb inner ----
                yaccs = [yaccp.tile([128, L], F16, tag=f"yacc{cb}", name=f"yacc_{seq}_{cb}")
                         for cb in range(NCB)]
                for qg in range(NQG):
                    bt = bcp.tile([128, SG * L], F16, tag="bbc", name=f"bbc_{seq}_{qg}")
                    ct = bcp.tile([128, SG * L], F16, tag="cbc", name=f"cbc_{seq}_{qg}")
                    nc.sync.dma_start(bt[:, :], _bcast_ap(bcd[qg * SG:(qg + 1) * SG, :], SG * L))
                    nc.sync.dma_start(ct[:, :], _bcast_ap(bcd[16 + qg * SG:16 + (qg + 1) * SG, :], SG * L))
                    for cb in range(NCB):
                        dt16r = reload_p.tile([128, L], F16, tag="dt16r", name=f"dt16r_{seq}_{qg}_{cb}")
                        nc.sync.dma_start(dt16r[:, :], dtD[cb][:, :])
                        dtx16r = reload_p.tile([128, L], F16, tag="dtx16r", name=f"dtx16r_{seq}_{qg}_{cb}")
                        nc.sync.dma_start(dtx16r[:, :], dtxD[cb][:, :])
                        dA16 = scan2.tile([128, SG * L], F16, tag="dA16", name=f"dA_{seq}_{qg}_{cb}")
                        for si in range(SG):
                            s = qg * SG + si
                            nc.scalar.activation(dA16[:, si * L:(si + 1) * L], dt16r[:, :],
                                                 AF.Exp, bias=0.0,
                                                 scale=A_sb[:, cb * D_STATE + s: cb * D_STATE + s + 1])
                        dA3 = dA16[:, :].rearrange("p (s t) -> p s t", s=SG)
                        nc.gpsimd.memset(dA3[:, :, 0:1], 0.0)
                        u16 = scan1.tile([128, SG * L], F16, tag="u16", name=f"u_{seq}_{qg}_{cb}")
                        u3 = u16[:, :].rearrange("p (s t) -> p s t", s=SG)
                        b3 = bt[:, :].rearrange("p (s t) -> p s t", s=SG)
                        dtxb = bass.AP(tensor=dtx16r[:, :].tensor, offset=dtx16r[:, :].offset,
                                       ap=[dtx16r[:, :].ap[0], [0, SG], [1, L]])
                        nc.vector.tensor_tensor(u3, dtxb, b3, OP.mult)
                        h16 = scan1.tile([128, SG * L], F16, tag="h16", name=f"h_{seq}_{qg}_{cb}")
                        nc.vector.tensor_tensor_scan(h16[:, :], dA16[:, :], u16[:, :], 0.0,
                                                     OP.mult, OP.add)
                        nc.vector.tensor_tensor(h16[:, :], h16[:, :], ct[:, :], OP.mult)
                        h3 = h16[:, :].rearrange("p (s t) -> p s t", s=SG)
                        nc.vector.tensor_tensor(h3[:, 0:SG // 2, :], h3[:, 0:SG // 2, :],
                                                h3[:, SG // 2:SG, :], OP.add)
                        if qg == 0:
                            nc.vector.tensor_tensor(yaccs[cb][:, :], h3[:, 0, :], h3[:, 1, :], OP.add)
                        else:
                            qs16 = yp.tile([128, L], F16, tag="qs16", name=f"qs_{seq}_{qg}_{cb}")
                            nc.vector.tensor_tensor(qs16[:, :], h3[:, 0, :], h3[:, 1, :], OP.add)
                            nc.vector.tensor_tensor(yaccs[cb][:, :], yaccs[cb][:, :], qs16[:, :], OP.add)

                # ---- tail: y2, z gate, y3 ----
                y3s = []
                for cb in range(NCB):
                    xs16r = reload_p.tile([128, L], F16, tag="xs16r", name=f"xs16r_{seq}_{cb}")
                    nc.sync.dma_start(xs16r[:, :], xsD[cb][:, :])
                    y2_16 = yp.tile([128, L], F16, tag="y2_16", name=f"y2_{seq}_{cb}")
                    nc.vector.scalar_tensor_tensor(y2_16[:, :], xs16r[:, :], d_sb[:, cb:cb + 1],
                                                   yaccs[cb][:, :], OP.mult, OP.add)
                    zth = tailp.tile([128, L], F16, tag="zth", name=f"zth_{seq}_{cb}")
                    z16 = tailp.tile([128, L], F16, tag="z16", name=f"z16_{seq}_{cb}")
                    for chk in range(NCH):
                        zps = ps_mm.tile([128, TCH], F32, tag="ps_mm", name=f"zps_{seq}_{cb}_{chk}")
                        for k in range(NKM):
                            nc.tensor.matmul(zps[:, :], lhs_in(k, NCB + cb),
                                             xT_sb[:, k * L + chk * TCH: k * L + (chk + 1) * TCH],
                                             start=(k == 0), stop=(k == NKM - 1))
                        nc.scalar.activation(zth[:, chk * TCH:(chk + 1) * TCH], zps[:, :],
                                             AF.Tanh, bias=0.0, scale=0.5)
                        nc.scalar.copy(z16[:, chk * TCH:(chk + 1) * TCH], zps[:, :])
                    nc.gpsimd.tensor_scalar(zth[:, :], zth[:, :], 0.5, 0.5, OP.mult, OP.add)
                    zz16 = tailp.tile([128, L], F16, tag="zz16", name=f"zz_{seq}_{cb}")
                    nc.gpsimd.tensor_tensor(zz16[:, :], z16[:, :], zth[:, :], OP.mult)
                    # y3 overwrites the (now dead) yacc tile to save SBUF
                    y3_16 = yaccs[cb]
                    nc.vector.tensor_tensor(y3_16[:, :], y2_16[:, :], zz16[:, :], OP.mult)
                    y3s.append(y3_16)

                # ---- out_proj ----
                for chk in range(NCH):
                    for m in range(D_MODEL // 128):
                        ops = ps_o.tile([128, TCH], F32, tag="ps_o", name=f"ops_{seq}_{chk}_{m}")
                        for cb in range(NCB):
                            nc.tensor.matmul(
                                ops[:, :],
                                wout_sb[:, cb * D_MODEL + m * 128: cb * D_MODEL + m * 128 + 128],
                                y3s[cb][:, chk * TCH:(chk + 1) * TCH],
                                start=(cb == 0), stop=(cb == NCB - 1))
                        osb = tailp.tile([128, TCH], F32, tag="osb", name=f"osb_{seq}_{chk}_{m}")
                        nc.scalar.copy(osb[:, :], ops[:, :])
                        nc.sync.dma_start(outT[seq, m * 128:(m + 1) * 128,
                                               chk * TCH:(chk + 1) * TCH], osb[:, :])

    nc.compile()
    return nc


def prep_host_inputs(x, W_in, conv_w, conv_b, W_xproj, W_dt, b_dt, A_log, D, W_out,
                     nseq=NSEQ):
    x = np.asarray(x, np.float32)
    b, c, Lx, dm = x.shape
    xs = x.reshape(b * c, Lx, dm)
    W_in = np.asarray(W_in, np.float32)
    conv_w = np.asarray(conv_w, np.float32)
    conv_b = np.asarray(conv_b, np.float32)
    W_xproj = np.asarray(W_xproj, np.float32)
    W_dt = np.asarray(W_dt, np.float32)
    b_dt = np.asarray(b_dt, np.float32)
    A_log = np.asarray(A_log, np.float32)
    D = np.asarray(D, np.float32)
    W_out = np.asarray(W_out, np.float32)

    def ktile(wt, nk):  # (K, M) -> (128, nk*M) with k-tiles side by side
        K, M = wt.shape
        return np.ascontiguousarray(
            wt.reshape(nk, 128, M).transpose(1, 0, 2).reshape(128, nk * M))

    winT16 = ktile(W_in.T, NKM).astype(np.float16)
    wxT16 = ktile(W_xproj.T, NCB).astype(np.float16)
    wdtT16 = np.ascontiguousarray(W_dt.T).astype(np.float16)
    woutT16 = ktile(W_out.T, NCB).astype(np.float16)
    cdiag = np.zeros((128, D_CONV, NCB, 128), np.float32)
    p = np.arange(128)
    for j in range(D_CONV):
        for cb in range(NCB):
            cdiag[p, j, cb, p] = conv_w[cb * 128 + p, j]
    cdiag = cdiag.reshape(128, D_CONV * NCB * 128)
    A = -np.exp(A_log)
    Acol = A.reshape(NCB, 128, D_STATE).transpose(1, 0, 2).reshape(128, NCB * D_STATE)
    smallw = np.concatenate([
        Acol,
        b_dt.reshape(NCB, 128).T,
        0.5 * conv_b.reshape(NCB, 128).T,
        conv_b.reshape(NCB, 128).T,
        D.reshape(NCB, 128).T,
    ], axis=1).astype(np.float32)
    smallw = np.ascontiguousarray(smallw)

    shared = dict(winT16=winT16, wxT16=wxT16, wdtT16=wdtT16, woutT16=woutT16,
                  cdiag=cdiag, smallw=smallw)
    in_maps = []
    ncores = (b * c) // nseq
    for k in range(ncores):
        xk = xs[k * nseq:(k + 1) * nseq]
        xTk = xk.transpose(0, 2, 1)                          # (nseq, dm, L)
        xTk = xTk.reshape(nseq, NKM, 128, Lx).transpose(0, 2, 1, 3)
        xTk = np.ascontiguousarray(xTk.reshape(nseq, 128, NKM * Lx)).astype(np.float16)
        m = dict(shared)
        m["xT16"] = xTk
        in_maps.append(m)
    return in_maps


def _install_ntff_hook_shim():
    """The agent image's antenv lacks axon_hooks; recreate it so trace=True
    captures NTFF profiles (exec_time_ns) through the axon .so."""
    import sys
    import types
    if "antenv.axon_hooks" in sys.modules:
        return
    from trn_agent_boot.trn_boot import _ntff_profile_via_ctypes
    hook = _ntff_profile_via_ctypes("/opt/axon/libaxon_pjrt.so")
    m = types.ModuleType("antenv.axon_hooks")
    m._hook = hook
    m.get_axon_ntff_profile_hook = lambda: m._hook

    def _set(h):
        m._hook = h
    m.set_axon_ntff_profile_hook = _set
    sys.modules["antenv.axon_hooks"] = m
    import antenv
    antenv.axon_hooks = m


_NC_CACHE = {}


def kernel(**inputs):
    global LAST_EXEC_NS, LAST_PROFILE
    key = (NSEQ, SEQ_L)
    if key not in _NC_CACHE:
        _NC_CACHE[key] = build_program(NSEQ, SEQ_L)
    nc = _NC_CACHE[key]
    in_maps = prep_host_inputs(**inputs)
    trace = bool(int(os.environ.get("MAMBA_TRACE", "0")))
    if trace:
        _install_ntff_hook_shim()
    res = run_bass_kernel_spmd(nc, in_maps, core_ids=list(range(N_CORES)),
                               trace=trace)
    LAST_EXEC_NS = res.exec_time_ns
    LAST_PROFILE = res.profile_json
    x = inputs["x"]
    b, c, L, dm = np.asarray(x).shape
    out = np.empty((b * c, L, dm), np.float32)
    for k in range(N_CORES):
        outT = res.results[k]["outT"]
        out[k * NSEQ:(k + 1) * NSEQ] = np.asarray(outT).transpose(0, 2, 1)
    return out.reshape(b, c, L, dm)
